# revision 46
# baseline (speedup 1.0000x reference)
"""Trainium2 Bass kernel for ATP self-attention (B=2, S=2048, D=2048, H=16).

Strategy (8 NeuronCores, tensor-parallel over heads, 2 heads/core):
  Host stages inputs: x pre-transposed to xT [D, T] and cast to bf16,
  w_qkv column-shard per core reordered to [q0|k0|q1|k1|v0|v1] (bf16),
  w_dense bf16.
  phase 2: fused QKV projection: qT/kT in [wcol, tok] layout (w k-tiles
           stationary, xT moving) and v in natural [tok, vcol] layout
           (xT k-tiles stationary, w_v moving), bf16 in / fp32 PSUM.
  phase 3: per (batch, q-tile, local head): scoresT = kT-tile.T @ qT
           (kpos on psum partitions, qpos free), exp on ACT with 1/sqrt(hd)
           scale + attention-mask bias fused, causal via skipping
           strictly-upper k-tiles + 0/1 masks on diagonal blocks,
           denominator accumulated on the PE (ones-vector matmul),
           ctxT = v.T @ expT accumulated in PSUM, normalized with an
           fp32 broadcast reciprocal.
  AllToAll: core sends its ctxT columns per destination token block,
           receives full-D ctxT (bf16) for its own 512-token slice.
  phase 4: dense out_slice = ctxT_slice.T @ w_dense + b_dense (fp32 out).
Host gathers the 8 [512, D] output slices.
"""

import sys
import types

sys.path.insert(0, "/opt/trn_rl_repo")

import ml_dtypes
import numpy as np

import concourse.bacc as bacc
import concourse.mybir as mybir
import concourse.tile as tile

B, S, D, H = 2, 2048, 2048, 16
HD = D // H                    # 128
T = B * S                      # 4096 tokens
N_CORES = 8
TSL = T // N_CORES             # 512 tokens per core
HL = H // N_CORES              # 2 local heads
WQC = 3 * D // N_CORES         # 768 qkv columns per core
SCALE = 1.0 / float(np.sqrt(HD))

F32 = mybir.dt.float32
BF16 = mybir.dt.bfloat16
ADD = mybir.AluOpType.add
MULT = mybir.AluOpType.mult


def build(am_zero=True):
    nc = bacc.Bacc("TRN2", target_bir_lowering=False, debug=False,
                   num_devices=N_CORES)
    xT = nc.dram_tensor("xT", [D, T], BF16, kind="ExternalInput").ap()
    wq = nc.dram_tensor("wq", [D, WQC], BF16, kind="ExternalInput").ap()
    bqqk = nc.dram_tensor("bqqk", [512], F32, kind="ExternalInput").ap()
    bqv = nc.dram_tensor("bqv", [256], F32, kind="ExternalInput").ap()
    am = nc.dram_tensor("am", [B, S], F32, kind="ExternalInput").ap()
    wd = nc.dram_tensor("wd", [D, D], BF16, kind="ExternalInput").ap()
    bd = nc.dram_tensor("bd", [D], F32, kind="ExternalInput").ap()
    out = nc.dram_tensor("out", [TSL, D], F32, kind="ExternalOutput").ap()

    with tile.TileContext(nc) as tc:
        with tc.tile_pool(name="consts", bufs=1) as consts, \
             tc.tile_pool(name="qkvT", bufs=1) as qkvT_pool, \
             tc.tile_pool(name="vsb", bufs=1) as vsb_pool, \
             tc.tile_pool(name="dram", bufs=1, space="DRAM") as dram:

            # ---- constants ----
            # diagonal causal masks, fused in pairs: columns [1024*pp + 512*h]
            # hold mask for delta = 128*(2*pp + h):
            # mask[p, 512*t + c] = 1.0 if c - p - 128*t >= 0
            dmask = consts.tile([128, 2048], BF16, name="dmask")
            nc.gpsimd.memset(dmask[:], 1.0)
            for t in range(4):
                nc.gpsimd.affine_select(
                    out=dmask[:, 512 * t:512 * (t + 1)],
                    in_=dmask[:, 512 * t:512 * (t + 1)],
                    compare_op=mybir.AluOpType.is_ge,
                    fill=0.0,
                    base=-128 * t,
                    pattern=[[1, 512]],
                    channel_multiplier=-1,
                )

            ones_bf = consts.tile([128, 128], BF16)  # partition-sum+bcast lhsT
            nc.gpsimd.memset(ones_bf[:], 1.0)
            ones_row = consts.tile([1, 128], F32)    # broadcast lhsT
            nc.gpsimd.memset(ones_row[:], 1.0)

            # biases / attention mask staged per-partition
            bqqk_sb = consts.tile([128, 4], F32)
            nc.sync.dma_start(bqqk_sb[:], bqqk.rearrange("(o p) -> p o", p=128))
            am_sb = consts.tile([128, B * (S // 128)], F32)
            if not am_zero:
                nc.sync.dma_start(am_sb[:],
                                  am.rearrange("b (i p) -> p (b i)", p=128))
            bd_sb = consts.tile([1, D], F32)
            nc.sync.dma_start(bd_sb[:], bd[None, :])
            bqv_sb = consts.tile([1, 256], F32)
            nc.sync.dma_start(bqv_sb[:], bqv[None, :])

            # free-dim biases broadcast across partitions
            bd_rep = consts.tile([128, D], F32)
            bqv_rep2 = consts.tile([128, 512], F32)   # two copies side by side
            with tc.tile_pool(name="cpsum", bufs=2, space="PSUM") as cpsum:
                for n in range(4):
                    ps = cpsum.tile([128, 512], F32, name=f"bdps{n}", tag="bdps")
                    nc.tensor.matmul(ps[:], ones_row[:],
                                     bd_sb[:, 512 * n:512 * (n + 1)],
                                     start=True, stop=True)
                    nc.scalar.copy(bd_rep[:, 512 * n:512 * (n + 1)], ps[:])
                ps = cpsum.tile([128, 256], F32, name="bqvps", tag="bqvps")
                nc.tensor.matmul(ps[:], ones_row[:], bqv_sb[:], start=True, stop=True)
                nc.scalar.copy(bqv_rep2[:, 0:256], ps[:])
                nc.scalar.copy(bqv_rep2[:, 256:512], ps[:])

            # resident projections, per (c, token-block):
            # c order: q0, k0, q1, k1 (per local head, [wcol, tok] layout)
            qkvT = {(c, t): qkvT_pool.tile([128, 512], BF16, name=f"qkvT{c}_{t}")
                    for c in range(4) for t in range(T // 512)}
            # v in natural layout, two token tiles per sbuf tile:
            # v2_sb[gp][:, 256*h + vcol] = v[tok-tile 2*gp + h]
            v2_sb = [vsb_pool.tile([128, 512], BF16, name=f"v{gp}")
                     for gp in range(T // 256)]

            def v_lhsT(b, i, jh):
                g = 16 * b + i
                return v2_sb[g // 2][:, 256 * (g % 2) + 128 * jh:
                                     256 * (g % 2) + 128 * (jh + 1)]

            # DRAM bounce buffers for the two AllToAlls (one per local head)
            a2a_in = [dram.tile([N_CORES * HD, TSL], BF16, name=f"a2ain{j}")
                      for j in range(HL)]
            a2a_out = [dram.tile([N_CORES * HD, TSL], BF16, name=f"a2aout{j}")
                       for j in range(HL)]

            # tiny dummy collective to absorb the first-trigger wakeup cost
            # (overlaps with phase 2)
            warm_in = dram.tile([16, 16], F32)
            warm_out = dram.tile([N_CORES * 16, 16], F32)
            nc.gpsimd.collective_compute(
                "AllGather", mybir.AluOpType.bypass,
                replica_groups=[list(range(N_CORES))],
                ins=[warm_in.opt()], outs=[warm_out.opt()],
            )

            # ---- phase 2: projections ----
            with tc.tile_pool(name="ph2wq", bufs=1) as ph2wq, \
                 tc.tile_pool(name="ph2", bufs=4) as ph2, \
                 tc.tile_pool(name="ph2ps", bufs=1, space="PSUM") as ph2ps:
                # interleave weight loads with the first token block's xT loads
                # so the PE can start on (k=0) immediately
                wq_sb = []
                xr0 = []
                for k in range(16):
                    w_t = ph2wq.tile([128, WQC], BF16, name=f"wq{k}")
                    nc.sync.dma_start(w_t[:], wq[128 * k:128 * (k + 1), :])
                    wq_sb.append(w_t)
                    x_t = ph2wq.tile([128, 512], BF16, name=f"xr0_{k}")
                    nc.sync.dma_start(x_t[:], xT[128 * k:128 * (k + 1), 0:512])
                    xr0.append(x_t)
                for t in range(T // 512):
                    psq = [ph2ps.tile([128, 512], F32, name=f"psq{c}", tag=f"psq{c}")
                           for c in range(4)]
                    psv = [ph2ps.tile([128, 256], F32, name=f"psv{m}", tag=f"psv{m}")
                           for m in range(4)]
                    for k in range(16):
                        if t == 0:
                            xr = xr0[k]
                        else:
                            xr = ph2.tile([128, 512], BF16, name="xr", tag="xr")
                            nc.sync.dma_start(
                                xr[:],
                                xT[128 * k:128 * (k + 1), 512 * t:512 * (t + 1)])
                        for c in range(4):
                            nc.tensor.matmul(
                                psq[c][:], wq_sb[k][:, 128 * c:128 * (c + 1)], xr[:],
                                start=(k == 0), stop=(k == 15))
                        for m in range(4):
                            nc.tensor.matmul(
                                psv[m][:], xr[:, 128 * m:128 * (m + 1)],
                                wq_sb[k][:, 512:768],
                                start=(k == 0), stop=(k == 15))
                    for c in range(4):
                        nc.scalar.activation(
                            qkvT[(c, t)][:], psq[c][:],
                            mybir.ActivationFunctionType.Identity,
                            bias=bqqk_sb[:, c:c + 1])
                    for m in range(4):
                        nc.vector.tensor_tensor(
                            v2_sb[2 * t + m // 2][:, 256 * (m % 2):256 * (m % 2 + 1)],
                            psv[m][:], bqv_rep2[:, 0:256], ADD)

            # ---- phase 4 weight prefetch (streams during phase 3) ----
            ph4w = tc.alloc_tile_pool(name="ph4w", bufs=1)
            wd_sb = {}
            for k in range(16):
                for n in range(4):
                    w_t = ph4w.tile([128, 512], BF16, name=f"wd{k}_{n}")
                    nc.sync.dma_start(
                        w_t[:], wd[128 * k:128 * (k + 1), 512 * n:512 * (n + 1)])
                    wd_sb[(k, n)] = w_t

            # ---- phase 3: attention, software-pipelined over k-tile PAIRS ----
            # item = (jh, b, jq, p) covering k-tiles 2p, 2p+1; scores run LAG
            # items ahead of den/ctx; normalization deferred LAG_N items.
            # jh is the outer loop so AllToAll for jh=0 overlaps jh=1 compute.
            LAG, LAG_N = 3, 5
            items = []
            for jh in range(HL):
                for b in range(B):
                    for jq in range(4):
                        npair = 2 * jq + 2
                        for p in range(npair):
                            items.append((jh, b, jq, p, p == npair - 1))
            mmps = tc.alloc_tile_pool(name="mmps", bufs=1, space="PSUM")
            with tc.tile_pool(name="ph3", bufs=4) as ph3:
                state = {}   # (jh,b,jq) -> dict with psum tiles / e tiles
                pend_norm = []   # (emit_after_idx, group_key)

                def emit_scores(idx):
                    jh, b, jq, p, last = items[idx]
                    g = (jh, b, jq)
                    st = state.setdefault(g, {"e": {}})
                    if "ctx" not in st:
                        st["ctx"] = mmps.tile([128, 512], F32, name="ctxps",
                                              tag="ctxps", bufs=2)
                        st["drep"] = mmps.tile([128, 512], F32, name="denrep",
                                               tag="denrep", bufs=2)
                    qT_t = qkvT[(2 * jh, 4 * b + jq)]
                    s2 = mmps.tile([128, 1024], F32, name="sps", tag="sps",
                                   bufs=2)
                    for h in range(2):
                        i = 2 * p + h
                        kT_t = qkvT[(2 * jh + 1, 4 * b + i // 4)]
                        nc.tensor.matmul(
                            s2[:, 512 * h:512 * (h + 1)],
                            kT_t[:, 128 * (i % 4):128 * (i % 4 + 1)], qT_t[:],
                            start=True, stop=True)
                    e2 = ph3.tile([128, 1024], BF16, name="exp", tag="exp",
                                  bufs=6)
                    if am_zero:
                        nc.scalar.activation(
                            e2[:], s2[:], mybir.ActivationFunctionType.Exp,
                            scale=SCALE)
                    else:
                        for h in range(2):
                            i = 2 * p + h
                            nc.scalar.activation(
                                e2[:, 512 * h:512 * (h + 1)],
                                s2[:, 512 * h:512 * (h + 1)],
                                mybir.ActivationFunctionType.Exp,
                                bias=am_sb[:, b * 16 + i:b * 16 + i + 1],
                                scale=SCALE)
                    npair = 2 * jq + 2
                    if p >= npair - 2:
                        pp = p - (npair - 2)   # 0 or 1 within the diagonal
                        nc.vector.tensor_tensor(
                            e2[:], e2[:], dmask[:, 1024 * pp:1024 * (pp + 1)],
                            MULT)
                    st["e"][p] = e2

                def emit_denctx(idx):
                    jh, b, jq, p, last = items[idx]
                    g = (jh, b, jq)
                    st = state[g]
                    e2 = st["e"].pop(p)
                    npair = 2 * jq + 2
                    # pair-sum for the denominator (one DVE op), then
                    # replicated-denominator accumulation on the PE
                    dp = ph3.tile([128, 512], BF16, name="dp", tag="dp")
                    nc.vector.tensor_tensor(dp[:], e2[:, 0:512], e2[:, 512:1024],
                                            ADD)
                    nc.tensor.matmul(st["drep"][:], ones_bf[:], dp[:],
                                     start=(p == 0), stop=(p == npair - 1))
                    for h in range(2):
                        i = 2 * p + h
                        nc.tensor.matmul(
                            st["ctx"][:], v_lhsT(b, i, jh),
                            e2[:, 512 * h:512 * (h + 1)],
                            start=(i == 0), stop=(i == 2 * npair - 1))
                    if last:
                        pend_norm.append((idx + LAG_N, g))

                def emit_norm(g):
                    jh, b, jq = g
                    st = state.pop(g)
                    rcp = ph3.tile([128, 512], F32, name="rcp", tag="rcp")
                    nc.vector.reciprocal_approx_fast(rcp[:], st["drep"][:])
                    ctx_sb = ph3.tile([128, 512], BF16, name="ctxsb", tag="ctxsb")
                    nc.vector.tensor_tensor(ctx_sb[:], st["ctx"][:], rcp[:], MULT)
                    tb = 4 * b + jq
                    nc.gpsimd.dma_start(
                        a2a_in[jh][128 * tb:128 * (tb + 1), :], ctx_sb[:])

                n_items = len(items)
                half = n_items // 2
                for idx in range(n_items + LAG):
                    if idx < n_items:
                        emit_scores(idx)
                    if idx >= LAG:
                        emit_denctx(idx - LAG)
                    while pend_norm and pend_norm[0][0] <= idx:
                        emit_norm(pend_norm.pop(0)[1])
                    if idx == half + LAG_N + 1:
                        # all jh=0 groups are normalized by now; flush + launch
                        while pend_norm and pend_norm[0][1][0] == 0:
                            emit_norm(pend_norm.pop(0)[1])
                        nc.gpsimd.collective_compute(
                            "AllToAll", mybir.AluOpType.bypass,
                            replica_groups=[list(range(N_CORES))],
                            ins=[a2a_in[0].opt()], outs=[a2a_out[0].opt()],
                        )
                while pend_norm:
                    emit_norm(pend_norm.pop(0)[1])

            nc.gpsimd.collective_compute(
                "AllToAll", mybir.AluOpType.bypass,
                replica_groups=[list(range(N_CORES))],
                ins=[a2a_in[1].opt()], outs=[a2a_out[1].opt()],
            )

            # ---- phase 4: dense on my token slice, two stages ----
            # stage A (jh=0 / even ctx col-tiles) reuses "sps" PSUM slots so
            # it can start while the tail of phase 3 still runs; stage B waits
            # for AllToAll #2.
            with tc.tile_pool(name="ph4ct", bufs=1) as ph4ct, \
                 tc.tile_pool(name="ph4pt", bufs=1) as ph4pt, \
                 tc.tile_pool(name="ph4", bufs=3) as ph4:
                ct = {}
                for jh in range(HL):
                    for r in range(N_CORES):
                        c_t = ph4ct.tile([128, 512], BF16, name=f"ct{jh}_{r}")
                        nc.sync.dma_start(
                            c_t[:], a2a_out[jh][128 * r:128 * (r + 1), :])
                        ct[(jh, r)] = c_t
                partial = {}
                for n in range(4):
                    for m in range(4):
                        ps = mmps.tile(
                            [128, 512], F32, name=f"opsA{n}_{m}",
                            tag=("denrep" if (4 * n + m) % 2 else "sps"), bufs=2)
                        for r in range(N_CORES):
                            nc.tensor.matmul(
                                ps[:], ct[(0, r)][:, 128 * m:128 * (m + 1)],
                                wd_sb[(2 * r, n)][:],
                                start=(r == 0), stop=(r == N_CORES - 1))
                        pt = ph4pt.tile([128, 512], F32, name=f"pt{n}_{m}")
                        nc.vector.tensor_tensor(
                            pt[:], ps[:], bd_rep[:, 512 * n:512 * (n + 1)], ADD)
                        partial[(n, m)] = pt
                stageb_tags = ["sps", "sps", "ctxps", "ctxps"]
                for n in range(4):
                    ps = [mmps.tile([128, 512], F32, name=f"opsB{n}_{m}",
                                    tag=stageb_tags[m], bufs=2)
                          for m in range(4)]
                    for r in range(N_CORES):
                        for m in range(4):
                            nc.tensor.matmul(
                                ps[m][:], ct[(1, r)][:, 128 * m:128 * (m + 1)],
                                wd_sb[(2 * r + 1, n)][:],
                                start=(r == 0), stop=(r == N_CORES - 1))
                    for m in range(4):
                        ob = ph4.tile([128, 512], F32, name="ob", tag="ob")
                        nc.vector.tensor_tensor(
                            ob[:], ps[m][:], partial[(n, m)][:], ADD)
                        nc.sync.dma_start(
                            out[128 * m:128 * (m + 1), 512 * n:512 * (n + 1)],
                            ob[:])
            mmps.release()
            ph4w.release()

    nc.compile()
    return nc


_NC = {}


def _get_nc(am_zero=True):
    if am_zero not in _NC:
        _NC[am_zero] = build(am_zero)
    return _NC[am_zero]


def _install_ntff_hook():
    try:
        import antenv
        if "antenv.axon_hooks" in sys.modules:
            return
        mod = types.ModuleType("antenv.axon_hooks")
        mod._hook = None
        mod.set_axon_ntff_profile_hook = lambda h: setattr(mod, "_hook", h)
        mod.get_axon_ntff_profile_hook = lambda: mod._hook
        sys.modules["antenv.axon_hooks"] = mod
        antenv.axon_hooks = mod
        from trn_agent_boot.trn_boot import _ntff_profile_via_ctypes
        hook = _ntff_profile_via_ctypes("/opt/axon/libaxon_pjrt.so")
        if hook is not None:
            mod.set_axon_ntff_profile_hook(hook)
    except Exception:
        pass


def kernel(x, attention_mask, w_qkv, b_qkv, w_dense, b_dense, profile=False):
    import concourse.bass_utils as bass_utils
    if profile:
        _install_ntff_hook()
    amf0 = np.asarray(attention_mask, dtype=np.float32)
    nc = _get_nc(am_zero=not np.any(amf0))
    xf = np.asarray(x, dtype=np.float32).reshape(T, D)
    xTf = np.ascontiguousarray(xf.T).astype(ml_dtypes.bfloat16)
    amf = np.ascontiguousarray(
        np.asarray(attention_mask, dtype=np.float32).reshape(B, S))
    wqf = np.asarray(w_qkv, dtype=np.float32)
    bqf = np.asarray(b_qkv, dtype=np.float32)
    wdf = np.ascontiguousarray(
        np.asarray(w_dense, dtype=np.float32)).astype(ml_dtypes.bfloat16)
    bdf = np.ascontiguousarray(np.asarray(b_dense, dtype=np.float32))
    in_maps = []
    for r in range(N_CORES):
        # head h occupies w_qkv cols [384h, 384h+384) as [q|k|v];
        # reorder this core's shard to [q0|k0|q1|k1|v0|v1]
        h0, h1 = 2 * r, 2 * r + 1
        blocks = {}
        for tag, h in (("0", h0), ("1", h1)):
            base = 384 * h
            blocks["q" + tag] = (base, base + 128)
            blocks["k" + tag] = (base + 128, base + 256)
            blocks["v" + tag] = (base + 256, base + 384)
        order = ["q0", "k0", "q1", "k1", "v0", "v1"]
        wq_r = np.concatenate([wqf[:, blocks[o][0]:blocks[o][1]] for o in order],
                              axis=1)
        bq_r = np.concatenate([bqf[blocks[o][0]:blocks[o][1]] for o in order])
        in_maps.append({
            "xT": xTf,
            "wq": np.ascontiguousarray(wq_r).astype(ml_dtypes.bfloat16),
            "bqqk": np.ascontiguousarray(bq_r[:512]),
            "bqv": np.ascontiguousarray(bq_r[512:]),
            "am": amf,
            "wd": wdf,
            "bd": bdf,
        })
    res = bass_utils.run_bass_kernel_spmd(
        nc, in_maps, core_ids=list(range(N_CORES)), trace=profile)
    kernel.last_result = res
    full = np.concatenate([res.results[r]["out"] for r in range(N_CORES)], axis=0)
    return full.reshape(B, S, D).astype(np.float32, copy=False)


# revision 47
# speedup vs baseline: 1.0315x; 1.0315x over previous
"""Trainium2 Bass kernel for ATP self-attention (B=2, S=2048, D=2048, H=16).

Strategy (8 NeuronCores, tensor-parallel over heads, 2 heads/core):
  Host stages inputs: x pre-transposed to xT [D, T] and cast to bf16,
  w_qkv column-shard per core reordered to [q0|k0|q1|k1|v0|v1] (bf16),
  w_dense bf16.
  phase 2: fused QKV projection: qT/kT in [wcol, tok] layout (w k-tiles
           stationary, xT moving) and v in natural [tok, vcol] layout
           (xT k-tiles stationary, w_v moving), bf16 in / fp32 PSUM.
  phase 3: per (batch, q-tile, local head): scoresT = kT-tile.T @ qT
           (kpos on psum partitions, qpos free), exp on ACT with 1/sqrt(hd)
           scale + attention-mask bias fused, causal via skipping
           strictly-upper k-tiles + 0/1 masks on diagonal blocks,
           denominator accumulated on the PE (ones-vector matmul),
           ctxT = v.T @ expT accumulated in PSUM, normalized with an
           fp32 broadcast reciprocal.
  AllToAll: core sends its ctxT columns per destination token block,
           receives full-D ctxT (bf16) for its own 512-token slice.
  phase 4: dense out_slice = ctxT_slice.T @ w_dense + b_dense (fp32 out).
Host gathers the 8 [512, D] output slices.
"""

import sys
import types

sys.path.insert(0, "/opt/trn_rl_repo")

import ml_dtypes
import numpy as np

import concourse.bacc as bacc
import concourse.mybir as mybir
import concourse.tile as tile

B, S, D, H = 2, 2048, 2048, 16
HD = D // H                    # 128
T = B * S                      # 4096 tokens
N_CORES = 8
TSL = T // N_CORES             # 512 tokens per core
HL = H // N_CORES              # 2 local heads
WQC = 3 * D // N_CORES         # 768 qkv columns per core
SCALE = 1.0 / float(np.sqrt(HD))

F32 = mybir.dt.float32
BF16 = mybir.dt.bfloat16
ADD = mybir.AluOpType.add
MULT = mybir.AluOpType.mult


def build(am_zero=True):
    nc = bacc.Bacc("TRN2", target_bir_lowering=False, debug=False,
                   num_devices=N_CORES)
    xT = nc.dram_tensor("xT", [D, T], BF16, kind="ExternalInput").ap()
    wq = nc.dram_tensor("wq", [D, WQC], BF16, kind="ExternalInput").ap()
    bqqk = nc.dram_tensor("bqqk", [512], F32, kind="ExternalInput").ap()
    bqv = nc.dram_tensor("bqv", [256], F32, kind="ExternalInput").ap()
    am = nc.dram_tensor("am", [B, S], F32, kind="ExternalInput").ap()
    wd = nc.dram_tensor("wd", [D, D], BF16, kind="ExternalInput").ap()
    bd = nc.dram_tensor("bd", [D], F32, kind="ExternalInput").ap()
    out = nc.dram_tensor("out", [TSL, D], F32, kind="ExternalOutput").ap()

    with tile.TileContext(nc) as tc:
        with tc.tile_pool(name="consts", bufs=1) as consts, \
             tc.tile_pool(name="qkvT", bufs=1) as qkvT_pool, \
             tc.tile_pool(name="vsb", bufs=1) as vsb_pool, \
             tc.tile_pool(name="dram", bufs=1, space="DRAM") as dram:

            # ---- constants ----
            # diagonal causal masks, fused in pairs: columns [1024*pp + 512*h]
            # hold mask for delta = 128*(2*pp + h):
            # mask[p, 512*t + c] = 1.0 if c - p - 128*t >= 0
            dmask = consts.tile([128, 2048], BF16, name="dmask")
            nc.gpsimd.memset(dmask[:], 1.0)
            for t in range(4):
                nc.gpsimd.affine_select(
                    out=dmask[:, 512 * t:512 * (t + 1)],
                    in_=dmask[:, 512 * t:512 * (t + 1)],
                    compare_op=mybir.AluOpType.is_ge,
                    fill=0.0,
                    base=-128 * t,
                    pattern=[[1, 512]],
                    channel_multiplier=-1,
                )

            ones_bf = consts.tile([128, 128], BF16)  # partition-sum+bcast lhsT
            nc.gpsimd.memset(ones_bf[:], 1.0)
            ones_row = consts.tile([1, 128], F32)    # broadcast lhsT
            nc.gpsimd.memset(ones_row[:], 1.0)

            # biases / attention mask staged per-partition
            bqqk_sb = consts.tile([128, 4], F32)
            nc.sync.dma_start(bqqk_sb[:], bqqk.rearrange("(o p) -> p o", p=128))
            am_sb = consts.tile([128, B * (S // 128)], F32)
            if not am_zero:
                nc.sync.dma_start(am_sb[:],
                                  am.rearrange("b (i p) -> p (b i)", p=128))
            bd_sb = consts.tile([1, D], F32)
            nc.sync.dma_start(bd_sb[:], bd[None, :])
            bqv_sb = consts.tile([1, 256], F32)
            nc.sync.dma_start(bqv_sb[:], bqv[None, :])

            # free-dim biases broadcast across partitions
            bd_rep = consts.tile([128, D], F32)
            bqv_rep2 = consts.tile([128, 512], F32)   # two copies side by side
            with tc.tile_pool(name="cpsum", bufs=2, space="PSUM") as cpsum:
                for n in range(4):
                    ps = cpsum.tile([128, 512], F32, name=f"bdps{n}", tag="bdps")
                    nc.tensor.matmul(ps[:], ones_row[:],
                                     bd_sb[:, 512 * n:512 * (n + 1)],
                                     start=True, stop=True)
                    nc.scalar.copy(bd_rep[:, 512 * n:512 * (n + 1)], ps[:])
                ps = cpsum.tile([128, 256], F32, name="bqvps", tag="bqvps")
                nc.tensor.matmul(ps[:], ones_row[:], bqv_sb[:], start=True, stop=True)
                nc.scalar.copy(bqv_rep2[:, 0:256], ps[:])
                nc.scalar.copy(bqv_rep2[:, 256:512], ps[:])

            # resident projections, per (c, token-block):
            # c order: q0, k0, q1, k1 (per local head, [wcol, tok] layout)
            qkvT = {(c, t): qkvT_pool.tile([128, 512], BF16, name=f"qkvT{c}_{t}")
                    for c in range(4) for t in range(T // 512)}
            # v in natural layout, two token tiles per sbuf tile:
            # v2_sb[gp][:, 256*h + vcol] = v[tok-tile 2*gp + h]
            v2_sb = [vsb_pool.tile([128, 512], BF16, name=f"v{gp}")
                     for gp in range(T // 256)]

            def v_lhsT(b, i, jh):
                g = 16 * b + i
                return v2_sb[g // 2][:, 256 * (g % 2) + 128 * jh:
                                     256 * (g % 2) + 128 * (jh + 1)]

            # DRAM bounce buffers for the two AllToAlls (one per local head)
            a2a_in = [dram.tile([N_CORES * HD, TSL], BF16, name=f"a2ain{j}")
                      for j in range(HL)]
            a2a_out = [dram.tile([N_CORES * HD, TSL], BF16, name=f"a2aout{j}")
                       for j in range(HL)]

            # tiny dummy collective to absorb the first-trigger wakeup cost
            # (overlaps with phase 2)
            warm_in = dram.tile([16, 16], F32)
            warm_out = dram.tile([N_CORES * 16, 16], F32)
            nc.gpsimd.collective_compute(
                "AllGather", mybir.AluOpType.bypass,
                replica_groups=[list(range(N_CORES))],
                ins=[warm_in.opt()], outs=[warm_out.opt()],
            )

            # ---- phase 2: projections ----
            with tc.tile_pool(name="ph2wq", bufs=1) as ph2wq, \
                 tc.tile_pool(name="ph2", bufs=4) as ph2, \
                 tc.tile_pool(name="ph2ps", bufs=1, space="PSUM") as ph2ps:
                # interleave weight loads with the first token block's xT loads
                # so the PE can start on (k=0) immediately
                wq_sb = []
                xr0 = []
                for k in range(16):
                    w_t = ph2wq.tile([128, WQC], BF16, name=f"wq{k}")
                    nc.sync.dma_start(w_t[:], wq[128 * k:128 * (k + 1), :])
                    wq_sb.append(w_t)
                    x_t = ph2wq.tile([128, 512], BF16, name=f"xr0_{k}")
                    nc.sync.dma_start(x_t[:], xT[128 * k:128 * (k + 1), 0:512])
                    xr0.append(x_t)
                for t in range(T // 512):
                    psq = [ph2ps.tile([128, 512], F32, name=f"psq{c}", tag=f"psq{c}")
                           for c in range(4)]
                    psv = [ph2ps.tile([128, 256], F32, name=f"psv{m}", tag=f"psv{m}")
                           for m in range(4)]
                    for k in range(16):
                        if t == 0:
                            xr = xr0[k]
                        else:
                            xr = ph2.tile([128, 512], BF16, name="xr", tag="xr")
                            nc.sync.dma_start(
                                xr[:],
                                xT[128 * k:128 * (k + 1), 512 * t:512 * (t + 1)])
                        for c in range(4):
                            nc.tensor.matmul(
                                psq[c][:], wq_sb[k][:, 128 * c:128 * (c + 1)], xr[:],
                                start=(k == 0), stop=(k == 15))
                        for m in range(4):
                            nc.tensor.matmul(
                                psv[m][:], xr[:, 128 * m:128 * (m + 1)],
                                wq_sb[k][:, 512:768],
                                start=(k == 0), stop=(k == 15))
                    for c in range(4):
                        nc.scalar.activation(
                            qkvT[(c, t)][:], psq[c][:],
                            mybir.ActivationFunctionType.Identity,
                            bias=bqqk_sb[:, c:c + 1])
                    for m in range(4):
                        nc.vector.tensor_tensor(
                            v2_sb[2 * t + m // 2][:, 256 * (m % 2):256 * (m % 2 + 1)],
                            psv[m][:], bqv_rep2[:, 0:256], ADD)

            # ---- phase 4 weight prefetch (streams during phase 3) ----
            ph4w = tc.alloc_tile_pool(name="ph4w", bufs=1)
            wd_sb = {}
            for k in range(16):
                for n in range(4):
                    w_t = ph4w.tile([128, 512], BF16, name=f"wd{k}_{n}")
                    nc.sync.dma_start(
                        w_t[:], wd[128 * k:128 * (k + 1), 512 * n:512 * (n + 1)])
                    wd_sb[(k, n)] = w_t

            # ---- phase 3: attention, software-pipelined over k-tile PAIRS ----
            # item = (jh, b, jq, p) covering k-tiles 2p, 2p+1; scores run LAG
            # items ahead of den/ctx; normalization deferred LAG_N items.
            # jh is the outer loop so AllToAll for jh=0 overlaps jh=1 compute.
            LAG, LAG_N = 2, 4
            items = []
            for jh in range(HL):
                for b in range(B):
                    for jq in range(4):
                        npair = 2 * jq + 2
                        for p in range(npair):
                            items.append((jh, b, jq, p, p == npair - 1))
            mmps = tc.alloc_tile_pool(name="mmps", bufs=1, space="PSUM")
            with tc.tile_pool(name="ph3", bufs=4) as ph3:
                state = {}   # (jh,b,jq) -> dict with psum tiles / e tiles
                pend_norm = []   # (emit_after_idx, group_key)

                def emit_scores(idx):
                    jh, b, jq, p, last = items[idx]
                    g = (jh, b, jq)
                    st = state.setdefault(g, {"e": {}})
                    if "ctx" not in st:
                        st["ctx"] = mmps.tile([128, 512], F32, name="ctxps",
                                              tag="ctxps", bufs=2)
                        st["drep"] = mmps.tile([128, 512], F32, name="denrep",
                                               tag="denrep", bufs=2)
                    qT_t = qkvT[(2 * jh, 4 * b + jq)]
                    s2 = mmps.tile([128, 1024], F32, name="sps", tag="sps",
                                   bufs=2)
                    for h in range(2):
                        i = 2 * p + h
                        kT_t = qkvT[(2 * jh + 1, 4 * b + i // 4)]
                        nc.tensor.matmul(
                            s2[:, 512 * h:512 * (h + 1)],
                            kT_t[:, 128 * (i % 4):128 * (i % 4 + 1)], qT_t[:],
                            start=True, stop=True)
                    e2 = ph3.tile([128, 1024], BF16, name="exp", tag="exp",
                                  bufs=4)
                    if am_zero:
                        nc.scalar.activation(
                            e2[:], s2[:], mybir.ActivationFunctionType.Exp,
                            scale=SCALE)
                    else:
                        for h in range(2):
                            i = 2 * p + h
                            nc.scalar.activation(
                                e2[:, 512 * h:512 * (h + 1)],
                                s2[:, 512 * h:512 * (h + 1)],
                                mybir.ActivationFunctionType.Exp,
                                bias=am_sb[:, b * 16 + i:b * 16 + i + 1],
                                scale=SCALE)
                    npair = 2 * jq + 2
                    if p >= npair - 2:
                        pp = p - (npair - 2)   # 0 or 1 within the diagonal
                        nc.vector.tensor_tensor(
                            e2[:], e2[:], dmask[:, 1024 * pp:1024 * (pp + 1)],
                            MULT)
                    st["e"][p] = e2

                def emit_denctx(idx):
                    jh, b, jq, p, last = items[idx]
                    g = (jh, b, jq)
                    st = state[g]
                    e2 = st["e"].pop(p)
                    npair = 2 * jq + 2
                    # pair-sum for the denominator (one DVE op), then
                    # replicated-denominator accumulation on the PE
                    dp = ph3.tile([128, 512], BF16, name="dp", tag="dp")
                    nc.vector.tensor_tensor(dp[:], e2[:, 0:512], e2[:, 512:1024],
                                            ADD)
                    nc.tensor.matmul(st["drep"][:], ones_bf[:], dp[:],
                                     start=(p == 0), stop=(p == npair - 1))
                    for h in range(2):
                        i = 2 * p + h
                        nc.tensor.matmul(
                            st["ctx"][:], v_lhsT(b, i, jh),
                            e2[:, 512 * h:512 * (h + 1)],
                            start=(i == 0), stop=(i == 2 * npair - 1))
                    if last:
                        pend_norm.append((idx + LAG_N, g))

                def emit_norm(g):
                    jh, b, jq = g
                    st = state.pop(g)
                    rcp = ph3.tile([128, 512], F32, name="rcp", tag="rcp")
                    nc.vector.reciprocal_approx_fast(rcp[:], st["drep"][:])
                    ctx_sb = ph3.tile([128, 512], BF16, name="ctxsb", tag="ctxsb")
                    nc.vector.tensor_tensor(ctx_sb[:], st["ctx"][:], rcp[:], MULT)
                    tb = 4 * b + jq
                    nc.gpsimd.dma_start(
                        a2a_in[jh][128 * tb:128 * (tb + 1), :], ctx_sb[:])

                n_items = len(items)
                half = n_items // 2
                for idx in range(n_items + LAG):
                    if idx < n_items:
                        emit_scores(idx)
                    if idx >= LAG:
                        emit_denctx(idx - LAG)
                    while pend_norm and pend_norm[0][0] <= idx:
                        emit_norm(pend_norm.pop(0)[1])
                    if idx == half + LAG_N + 1:
                        # all jh=0 groups are normalized by now; flush + launch
                        while pend_norm and pend_norm[0][1][0] == 0:
                            emit_norm(pend_norm.pop(0)[1])
                        nc.gpsimd.collective_compute(
                            "AllToAll", mybir.AluOpType.bypass,
                            replica_groups=[list(range(N_CORES))],
                            ins=[a2a_in[0].opt()], outs=[a2a_out[0].opt()],
                        )
                while pend_norm:
                    emit_norm(pend_norm.pop(0)[1])

            nc.gpsimd.collective_compute(
                "AllToAll", mybir.AluOpType.bypass,
                replica_groups=[list(range(N_CORES))],
                ins=[a2a_in[1].opt()], outs=[a2a_out[1].opt()],
            )

            # ---- phase 4: dense on my token slice, two stages ----
            # stage A (jh=0 / even ctx col-tiles) reuses "sps" PSUM slots so
            # it can start while the tail of phase 3 still runs; stage B waits
            # for AllToAll #2.
            with tc.tile_pool(name="ph4ct", bufs=1) as ph4ct, \
                 tc.tile_pool(name="ph4pt", bufs=1) as ph4pt, \
                 tc.tile_pool(name="ph4", bufs=3) as ph4:
                ct = {}
                for jh in range(HL):
                    for r in range(N_CORES):
                        c_t = ph4ct.tile([128, 512], BF16, name=f"ct{jh}_{r}")
                        nc.sync.dma_start(
                            c_t[:], a2a_out[jh][128 * r:128 * (r + 1), :])
                        ct[(jh, r)] = c_t
                partial = {}
                for n in range(4):
                    for m in range(4):
                        ps = mmps.tile(
                            [128, 512], F32, name=f"opsA{n}_{m}",
                            tag=("denrep" if (4 * n + m) % 2 else "sps"), bufs=2)
                        for r in range(N_CORES):
                            nc.tensor.matmul(
                                ps[:], ct[(0, r)][:, 128 * m:128 * (m + 1)],
                                wd_sb[(2 * r, n)][:],
                                start=(r == 0), stop=(r == N_CORES - 1))
                        pt = ph4pt.tile([128, 512], F32, name=f"pt{n}_{m}")
                        nc.vector.tensor_tensor(
                            pt[:], ps[:], bd_rep[:, 512 * n:512 * (n + 1)], ADD)
                        partial[(n, m)] = pt
                stageb_tags = ["sps", "sps", "ctxps", "ctxps"]
                for n in range(4):
                    ps = [mmps.tile([128, 512], F32, name=f"opsB{n}_{m}",
                                    tag=stageb_tags[m], bufs=2)
                          for m in range(4)]
                    for r in range(N_CORES):
                        for m in range(4):
                            nc.tensor.matmul(
                                ps[m][:], ct[(1, r)][:, 128 * m:128 * (m + 1)],
                                wd_sb[(2 * r + 1, n)][:],
                                start=(r == 0), stop=(r == N_CORES - 1))
                    for m in range(4):
                        ob = ph4.tile([128, 512], F32, name="ob", tag="ob")
                        nc.vector.tensor_tensor(
                            ob[:], ps[m][:], partial[(n, m)][:], ADD)
                        nc.sync.dma_start(
                            out[128 * m:128 * (m + 1), 512 * n:512 * (n + 1)],
                            ob[:])
            mmps.release()
            ph4w.release()

    nc.compile()
    return nc


_NC = {}


def _get_nc(am_zero=True):
    if am_zero not in _NC:
        _NC[am_zero] = build(am_zero)
    return _NC[am_zero]


def _install_ntff_hook():
    try:
        import antenv
        if "antenv.axon_hooks" in sys.modules:
            return
        mod = types.ModuleType("antenv.axon_hooks")
        mod._hook = None
        mod.set_axon_ntff_profile_hook = lambda h: setattr(mod, "_hook", h)
        mod.get_axon_ntff_profile_hook = lambda: mod._hook
        sys.modules["antenv.axon_hooks"] = mod
        antenv.axon_hooks = mod
        from trn_agent_boot.trn_boot import _ntff_profile_via_ctypes
        hook = _ntff_profile_via_ctypes("/opt/axon/libaxon_pjrt.so")
        if hook is not None:
            mod.set_axon_ntff_profile_hook(hook)
    except Exception:
        pass


def kernel(x, attention_mask, w_qkv, b_qkv, w_dense, b_dense, profile=False):
    import concourse.bass_utils as bass_utils
    if profile:
        _install_ntff_hook()
    amf0 = np.asarray(attention_mask, dtype=np.float32)
    nc = _get_nc(am_zero=not np.any(amf0))
    xf = np.asarray(x, dtype=np.float32).reshape(T, D)
    xTf = np.ascontiguousarray(xf.T).astype(ml_dtypes.bfloat16)
    amf = np.ascontiguousarray(
        np.asarray(attention_mask, dtype=np.float32).reshape(B, S))
    wqf = np.asarray(w_qkv, dtype=np.float32)
    bqf = np.asarray(b_qkv, dtype=np.float32)
    wdf = np.ascontiguousarray(
        np.asarray(w_dense, dtype=np.float32)).astype(ml_dtypes.bfloat16)
    bdf = np.ascontiguousarray(np.asarray(b_dense, dtype=np.float32))
    in_maps = []
    for r in range(N_CORES):
        # head h occupies w_qkv cols [384h, 384h+384) as [q|k|v];
        # reorder this core's shard to [q0|k0|q1|k1|v0|v1]
        h0, h1 = 2 * r, 2 * r + 1
        blocks = {}
        for tag, h in (("0", h0), ("1", h1)):
            base = 384 * h
            blocks["q" + tag] = (base, base + 128)
            blocks["k" + tag] = (base + 128, base + 256)
            blocks["v" + tag] = (base + 256, base + 384)
        order = ["q0", "k0", "q1", "k1", "v0", "v1"]
        wq_r = np.concatenate([wqf[:, blocks[o][0]:blocks[o][1]] for o in order],
                              axis=1)
        bq_r = np.concatenate([bqf[blocks[o][0]:blocks[o][1]] for o in order])
        in_maps.append({
            "xT": xTf,
            "wq": np.ascontiguousarray(wq_r).astype(ml_dtypes.bfloat16),
            "bqqk": np.ascontiguousarray(bq_r[:512]),
            "bqv": np.ascontiguousarray(bq_r[512:]),
            "am": amf,
            "wd": wdf,
            "bd": bdf,
        })
    res = bass_utils.run_bass_kernel_spmd(
        nc, in_maps, core_ids=list(range(N_CORES)), trace=profile)
    kernel.last_result = res
    full = np.concatenate([res.results[r]["out"] for r in range(N_CORES)], axis=0)
    return full.reshape(B, S, D).astype(np.float32, copy=False)
